# revision 7
# baseline (speedup 1.0000x reference)
"""Distributed GNN message-passing + cluster-loss kernel for 8 Trainium2 NeuronCores.

Sharding: src-node-partitioned. Each core owns 6250 src nodes, computes the
full X@W table (replicated), gathers xw[dst] rows for its edges (dma_gather,
3 SWDGE queues), segment-sums by src via host-precomputed one-hot matmuls in
PSUM, applies selu/softmax, AllGathers the assignment matrix S^T, and computes
the edge/cluster reductions with ap_gather + small matmuls. Final scalar is
AllReduced and assembled on-device.
"""
import os
import sys

sys.path.insert(0, "/opt/trn_rl_repo")

import numpy as np
import ml_dtypes

import concourse.bass as bass
import concourse.tile as tile
from concourse import bacc, mybir
from concourse.bass_utils import run_bass_kernel_spmd

BF16 = mybir.dt.bfloat16
FP32 = mybir.dt.float32
I16 = mybir.dt.int16

N = 50000
F = 512
H = 512
K = 16
E = 1600000
NCORES = 8
NS = N // NCORES            # 6250 src nodes per core
SB = 49                     # src blocks per core (48 full + 1 of 106)
LASTW = NS - 48 * 128       # 106
HALF1 = 25088               # xw table split (49*512, sub-block aligned)
W_LO = HALF1                # 25088 rows
W_HI = N - HALF1            # 24912 rows
LAM = 1.0507009873554805
ALPHA = 1.6732632423543772

bf16 = ml_dtypes.bfloat16

_last_results = None        # test harness reads exec_time from here


def _round_up(x, m):
    return (x + m - 1) // m * m


def _prep_inputs(src, dst, graph_vals, gnorm_vals, features, W, b, Wt, bt):
    """Host-side sharding: per-core edge layouts, one-hot tiles, idx arrays."""
    src = np.asarray(src)
    dst = np.asarray(dst)
    graph_vals = np.asarray(graph_vals, dtype=np.float32)
    gnorm_vals = np.asarray(gnorm_vals, dtype=np.float32)
    gv_ones = bool(np.all(graph_vals == 1.0))

    core_of = src // NS
    local = src - core_of * NS
    blk = np.minimum(local // 128, SB - 1)
    slocal = local - blk * 128
    half = (dst >= HALF1).astype(np.int64)
    seg = (core_of * SB + blk) * 2 + half          # 0 .. 8*SB*2-1
    order = np.lexsort((dst, seg))
    seg_s = seg[order]
    dst_s = dst[order]
    slocal_s = slocal[order]
    g_s = gnorm_vals[order]

    nseg = NCORES * SB * 2
    counts = np.bincount(seg_s, minlength=nseg)
    CH = max(18, int(np.ceil(counts.max() / 128)))
    seg_starts = np.zeros(nseg + 1, np.int64)
    np.cumsum(counts, out=seg_starts[1:])
    pos = np.arange(E, dtype=np.int64) - seg_starts[seg_s]

    segc_all = seg_s % (SB * 2)
    core_all = seg_s // (SB * 2)
    chunk_all = pos // 128
    e_in_all = pos % 128
    col_all = (segc_all * CH + chunk_all) * 128 + slocal_s
    idxv_all = (dst_s - (segc_all % 2) * HALF1).astype(np.int16)

    onehots, gidxs = [], []
    for c in range(NCORES):
        m = core_all == c
        oh = np.zeros((128, SB * 2 * CH * 128), bf16)
        oh[e_in_all[m], col_all[m]] = g_s[m].astype(bf16)
        onehots.append(oh)
        base16 = np.zeros((16, SB * 2 * CH * 8), np.int16)
        base16[pos[m] % 16, segc_all[m] * (CH * 8) + pos[m] // 16] = idxv_all[m]
        gidxs.append(np.tile(base16, (8, 1)))

    # ---- phase E layouts (per core, per dst-half) ----
    # first pass: find global PH (slots per 16-partition group)
    max_per_g = 0
    for c in range(NCORES):
        em = core_of == c
        d_e = dst[em]
        for hh in (0, 1):
            n = int(np.sum((d_e >= HALF1) == bool(hh)))
            max_per_g = max(max_per_g, -(-n // 8))
    PH = _round_up(max(max_per_g, 128), 128)

    pdsts, psrcs, pgs = [], [], []
    for c in range(NCORES):
        em = core_of == c
        d_e = dst[em]
        s_e = local[em]
        gv_e = graph_vals[em]
        pd_halves, ps_halves, pg_halves = [], [], []
        for hh in (0, 1):
            sel = (d_e >= HALF1) == bool(hh)
            d_h = (d_e[sel] - hh * HALF1).astype(np.int16)
            s_h = s_e[sel].astype(np.int16)
            g_h = gv_e[sel]
            n = len(d_h)
            wsent = (W_LO if hh == 0 else W_HI)
            dpad = np.full(8 * PH, wsent, np.int16)     # sentinel zero-col
            spad = np.full(8 * PH, NS, np.int16)
            gpad = np.zeros(8 * PH, np.float32)
            dpad[:n] = d_h
            spad[:n] = s_h
            gpad[:n] = g_h

            def wrap(a):
                return np.ascontiguousarray(
                    a.reshape(8, PH // 16, 16).transpose(0, 2, 1).reshape(128, PH // 16)
                )
            pd_halves.append(wrap(dpad))
            ps_halves.append(wrap(spad))
            pg_halves.append(np.ascontiguousarray(
                np.repeat(gpad.reshape(8, 1, PH), 16, axis=1).reshape(128, PH)))
        pdsts.append(pd_halves)
        psrcs.append(ps_halves)
        pgs.append(pg_halves)

    # ---- shared constant tensors ----
    ftT = np.ascontiguousarray(features.T).astype(bf16)            # [512, N]
    w_arr = np.ascontiguousarray(
        np.asarray(W).astype(bf16).reshape(4, 128, H).transpose(1, 0, 2).reshape(128, 4 * H))
    wt_arr = np.ascontiguousarray(
        np.asarray(Wt).astype(bf16).reshape(4, 128, K).transpose(1, 0, 2).reshape(128, 4 * K))
    b_bcast = np.tile(np.asarray(b, np.float32)[None, :], (128, 1))
    btv = np.asarray(bt, np.float32).reshape(K, 1)
    ident = np.eye(128, dtype=np.float32)
    identb = np.eye(128, dtype=bf16)
    ones128 = np.ones((128, 1), np.float32)
    grp16 = np.zeros((128, K), np.float32)
    grp16[np.arange(128), np.arange(128) % 16] = 1.0
    smask = np.ones((128, 1), np.float32)
    smask[LASTW:, 0] = 0.0

    shared = dict(ft=ftT, w=w_arr, wt=wt_arr, bb=b_bcast, btv=btv,
                  idf=ident, idb=identb, ones=ones128, grp=grp16, smask=smask)

    in_maps = []
    for c in range(NCORES):
        m = dict(shared)
        m["oh"] = onehots[c]
        m["gidx"] = gidxs[c]
        m["pdlo"] = pdsts[c][0]
        m["pdhi"] = pdsts[c][1]
        m["pslo"] = psrcs[c][0]
        m["pshi"] = psrcs[c][1]
        if not gv_ones:
            m["pglo"] = pgs[c][0]
            m["pghi"] = pgs[c][1]
        in_maps.append(m)
    return in_maps, CH, PH, gv_ones


def _build_module(CH, PH, gv_ones):
    nc = bacc.Bacc("TRN2", target_bir_lowering=False, debug=False,
                   enable_asserts=False, num_devices=NCORES,
                   num_swdge_queues=3)

    dt = nc.dram_tensor
    ft = dt("ft", [F, N], BF16, kind="ExternalInput").ap()
    w_in = dt("w", [128, 4 * H], BF16, kind="ExternalInput").ap()
    wt_in = dt("wt", [128, 4 * K], BF16, kind="ExternalInput").ap()
    bb_in = dt("bb", [128, H], FP32, kind="ExternalInput").ap()
    btv_in = dt("btv", [K, 1], FP32, kind="ExternalInput").ap()
    idf_in = dt("idf", [128, 128], FP32, kind="ExternalInput").ap()
    idb_in = dt("idb", [128, 128], BF16, kind="ExternalInput").ap()
    ones_in = dt("ones", [128, 1], FP32, kind="ExternalInput").ap()
    grp_in = dt("grp", [128, K], FP32, kind="ExternalInput").ap()
    smask_in = dt("smask", [128, 1], FP32, kind="ExternalInput").ap()
    oh_in = dt("oh", [128, SB * 2 * CH * 128], BF16, kind="ExternalInput").ap()
    gidx_in = dt("gidx", [128, SB * 2 * CH * 8], I16, kind="ExternalInput").ap()
    pdlo_in = dt("pdlo", [128, PH // 16], I16, kind="ExternalInput").ap()
    pdhi_in = dt("pdhi", [128, PH // 16], I16, kind="ExternalInput").ap()
    pslo_in = dt("pslo", [128, PH // 16], I16, kind="ExternalInput").ap()
    pshi_in = dt("pshi", [128, PH // 16], I16, kind="ExternalInput").ap()
    if not gv_ones:
        pglo_in = dt("pglo", [128, PH], FP32, kind="ExternalInput").ap()
        pghi_in = dt("pghi", [128, PH], FP32, kind="ExternalInput").ap()

    y_out = dt("y", [1, 1], FP32, kind="ExternalOutput").ap()

    xw_lo = dt("xw_lo", [W_LO, H], BF16, kind="Internal").ap()
    xw_hi = dt("xw_hi", [W_HI, H], BF16, kind="Internal").ap()
    cc_in = dt("cc_in", [K, NS], FP32, kind="Internal").ap()
    cc_out = dt("cc_out", [NCORES * K, NS], FP32, kind="Internal",
                addr_space="Shared").ap()
    cc2_in = dt("cc2_in", [1, 33], FP32, kind="Internal").ap()
    cc2_out = dt("cc2_out", [1, 33], FP32, kind="Internal",
                 addr_space="Shared").ap()

    rg = [list(range(NCORES))]
    ftr = ft.rearrange("(j p) n -> p j n", p=128)   # [128, 4, N]

    with tile.TileContext(nc) as tc:
        with (
            tc.tile_pool(name="const", bufs=1) as cpool,
            tc.tile_pool(name="stslice", bufs=1) as stpool,
            tc.tile_pool(name="accs", bufs=1) as apool,
        ):
            idf_sb = cpool.tile([128, 128], FP32)
            nc.sync.dma_start(idf_sb[:], idf_in[:])
            ones_sb = cpool.tile([128, 1], FP32)
            nc.sync.dma_start(ones_sb[:], ones_in[:])
            grp_sb = cpool.tile([128, K], FP32)
            nc.sync.dma_start(grp_sb[:], grp_in[:])
            smask_sb = cpool.tile([128, 1], FP32)
            nc.sync.dma_start(smask_sb[:], smask_in[:])

            st_sb = stpool.tile([K, NS], FP32)       # S^T slice for this core
            cs_acc = apool.tile([128, K], FP32)
            nc.vector.memset(cs_acc[:], 0.0)
            edge_acc = apool.tile([128, 1], FP32)
            nc.vector.memset(edge_acc[:], 0.0)
            nl_acc = apool.tile([128, 1], FP32)
            nc.vector.memset(nl_acc[:], 0.0)

            # ---------- Phase A: xw = X @ W (full table, bf16) ----------
            bigc = tc.tile_pool(name="bigc", bufs=1)
            bigp = bigc.__enter__()
            w_sb = bigp.tile([128, 4, H], BF16)
            nc.sync.dma_start(w_sb[:], w_in.rearrange("p (j h) -> p j h", j=4))
            wt_sb = bigp.tile([128, 4, K], BF16)
            nc.sync.dma_start(wt_sb[:], wt_in.rearrange("p (j k) -> p j k", j=4))
            bb_sb = bigp.tile([128, H], FP32)
            nc.sync.dma_start(bb_sb[:], bb_in[:])
            btv_sb = bigp.tile([K, 1], FP32)
            nc.sync.dma_start(btv_sb[:], btv_in[:])
            gidx_sb = bigp.tile([128, SB * 2 * CH * 8], I16)
            nc.sync.dma_start(gidx_sb[:], gidx_in[:])
            with (
                tc.tile_pool(name="ftp", bufs=3) as ftp,
                tc.tile_pool(name="psA", bufs=2, space="PSUM") as psA,
                tc.tile_pool(name="xws", bufs=2) as xwsp,
            ):
                for nsb in range(98):
                    n0 = nsb * 512
                    wdt = min(512, N - n0)
                    ftt = ftp.tile([128, 4, 512], BF16, tag="ftt")
                    nc.sync.dma_start(ftt[:, :, :wdt], ftr[:, :, n0:n0 + wdt])
                    xws = xwsp.tile([128, 4, H], BF16, tag="xws")
                    nsub = (wdt + 127) // 128
                    for sx in range(nsub):
                        mm = min(128, wdt - sx * 128)
                        ps = psA.tile([128, H], FP32, tag="psA")
                        for j in range(4):
                            nc.tensor.matmul(
                                ps[:mm, :],
                                ftt[:, j, sx * 128:sx * 128 + mm],
                                w_sb[:, j, :],
                                start=(j == 0), stop=(j == 3))
                        nc.scalar.activation(xws[:mm, sx, :], ps[:mm, :],
                                             mybir.ActivationFunctionType.Copy)
                        row0 = n0 + sx * 128
                        if row0 < HALF1:
                            dst_ap = xw_lo[row0:row0 + mm, :]
                        else:
                            dst_ap = xw_hi[row0 - HALF1:row0 - HALF1 + mm, :]
                        nc.sync.dma_start(dst_ap, xws[:mm, sx, :])

            tc.strict_bb_all_engine_barrier()

            # ---------- Phase B: SpMM + selu + logits + softmax ----------
            with (
                tc.tile_pool(name="gt", bufs=4) as gtp,
                tc.tile_pool(name="ohp", bufs=2) as ohp,
                tc.tile_pool(name="psP", bufs=2, space="PSUM") as psP,
                tc.tile_pool(name="psT", bufs=2, space="PSUM") as psT,
                tc.tile_pool(name="ps16", bufs=2, space="PSUM") as ps16,
                tc.tile_pool(name="psE", bufs=2, space="PSUM") as psE,
                tc.tile_pool(name="selu", bufs=2) as selup,
                tc.tile_pool(name="smx", bufs=2) as smxp,
            ):
                for bI in range(SB):
                    gts = []
                    for hh in range(2):
                        segi = bI * 2 + hh
                        gt = gtp.tile([128, CH, H], BF16, tag="gt")
                        nc.gpsimd.dma_gather(
                            gt[:], (xw_lo if hh == 0 else xw_hi)[:],
                            gidx_sb[:, segi * CH * 8:(segi + 1) * CH * 8],
                            CH * 128, CH * 128, H,
                            elem_step=H, single_packet=False,
                            queue_num=segi % 3)
                        gts.append(gt)
                    oht = ohp.tile([128, 2 * CH * 128], BF16, tag="oht")
                    nc.sync.dma_start(
                        oht[:], oh_in[:, bI * 2 * CH * 128:(bI + 1) * 2 * CH * 128])

                    ps = psP.tile([128, H], FP32, tag="psP")
                    for t in range(2 * CH):
                        gt = gts[t // CH]
                        cl = t % CH
                        nc.tensor.matmul(
                            ps[:], oht[:, t * 128:(t + 1) * 128],
                            gt[:, cl, :],
                            start=(t == 0), stop=(t == 2 * CH - 1))

                    # selu(prop + b)
                    tt = selup.tile([128, H], FP32, tag="t")
                    nc.vector.tensor_tensor(tt[:], ps[:], bb_sb[:],
                                            mybir.AluOpType.add)
                    ut = selup.tile([128, H], FP32, tag="u")
                    nc.scalar.activation(ut[:], tt[:],
                                         mybir.ActivationFunctionType.Relu)
                    mt = selup.tile([128, H], FP32, tag="m")
                    nc.vector.tensor_tensor(mt[:], tt[:], ut[:],
                                            mybir.AluOpType.subtract)
                    wt_t = selup.tile([128, H], FP32, tag="w")
                    nc.scalar.activation(wt_t[:], mt[:],
                                         mybir.ActivationFunctionType.Exp)
                    s5 = selup.tile([128, H], FP32, tag="s5")
                    nc.scalar.activation(s5[:], wt_t[:],
                                         mybir.ActivationFunctionType.Copy,
                                         bias=-LAM * ALPHA, scale=LAM * ALPHA)
                    g6 = selup.tile([128, H], FP32, tag="g6")
                    nc.scalar.activation(g6[:], ut[:],
                                         mybir.ActivationFunctionType.Copy,
                                         scale=LAM)
                    gcn = selup.tile([128, H], FP32, tag="gcn")
                    nc.vector.tensor_tensor(gcn[:], s5[:], g6[:],
                                            mybir.AluOpType.add)

                    # gcn^T via PE transposes
                    gctp = psT.tile([128, H], FP32, tag="psT")
                    for j in range(4):
                        nc.tensor.transpose(gctp[:, j * 128:(j + 1) * 128],
                                            gcn[:, j * 128:(j + 1) * 128],
                                            idf_sb[:])
                    gct = selup.tile([128, H], BF16, tag="gct")
                    nc.scalar.activation(gct[:], gctp[:],
                                         mybir.ActivationFunctionType.Copy)

                    # logits^T [16, 128]
                    psl = ps16.tile([K, 128], FP32, tag="p16")
                    for j in range(4):
                        nc.tensor.matmul(psl[:], wt_sb[:, j, :],
                                         gct[:, j * 128:(j + 1) * 128],
                                         start=(j == 0), stop=(j == 3))
                    expl = smxp.tile([K, 128], FP32, tag="expl")
                    nc.scalar.activation(expl[:], psl[:],
                                         mybir.ActivationFunctionType.Exp,
                                         bias=btv_sb[:])
                    explt = psE.tile([128, K], FP32, tag="psE")
                    nc.tensor.transpose(explt[:], expl[:], idf_sb[:16, :16])
                    den = smxp.tile([128, 1], FP32, tag="den")
                    nc.vector.tensor_reduce(den[:], explt[:],
                                            mybir.AxisListType.X,
                                            mybir.AluOpType.add)
                    rec = smxp.tile([128, 1], FP32, tag="rec")
                    nc.vector.reciprocal(rec[:], den[:])
                    s_b = smxp.tile([128, K], FP32, tag="sb")
                    nc.vector.tensor_scalar(s_b[:], explt[:], rec[:], None,
                                            mybir.AluOpType.mult)
                    if bI == SB - 1:
                        nc.vector.tensor_scalar(s_b[:], s_b[:], smask_sb[:],
                                                None, mybir.AluOpType.mult)
                    nc.vector.tensor_tensor(cs_acc[:], cs_acc[:], s_b[:],
                                            mybir.AluOpType.add)
                    stp = ps16.tile([K, 128], FP32, tag="p16")
                    nc.tensor.transpose(stp[:], s_b[:], idf_sb[:])
                    wv = 128 if bI < SB - 1 else LASTW
                    nc.scalar.activation(st_sb[:, bI * 128:bI * 128 + wv],
                                         stp[:, :wv],
                                         mybir.ActivationFunctionType.Copy)

            bigc.__exit__(None, None, None)

            # ---------- Phase C: AllGather S^T ----------
            nc.sync.dma_start(cc_in[:], st_sb[:])
            tc.strict_bb_all_engine_barrier()
            nc.gpsimd.collective_compute(
                "AllGather", mybir.AluOpType.bypass,
                ins=[cc_in[:]], outs=[cc_out[:]], replica_groups=rg)
            tc.strict_bb_all_engine_barrier()

            # ---------- Phase E: edge reductions ----------
            NQ_CALLS = 8
            PQ = PH // NQ_CALLS
            with (
                tc.tile_pool(name="tab", bufs=1) as tabp,
                tc.tile_pool(name="pidx", bufs=1) as pidxp,
                tc.tile_pool(name="gout", bufs=2) as goutp,
                tc.tile_pool(name="ework", bufs=2) as ewp,
            ):
                srctab = tabp.tile([128, NS + 1], FP32, tag="srctab")
                nc.vector.memset(srctab[:, NS:NS + 1], 0.0)
                # this core's own shard, replicated into 8 groups — use
                # partition_id-free approach: read from cc_in (local copy)
                for g in range(8):
                    nc.sync.dma_start(srctab[16 * g:16 * g + 16, 0:NS], cc_in[:])

                pidx_t = {}
                for nm, apin in (("pdlo", pdlo_in), ("pdhi", pdhi_in),
                                 ("pslo", pslo_in), ("pshi", pshi_in)):
                    t = pidxp.tile([128, PH // 16], I16, tag=nm)
                    nc.sync.dma_start(t[:], apin[:])
                    pidx_t[nm] = t
                if not gv_ones:
                    pg_t = {}
                    for nm, apin in (("pglo", pglo_in), ("pghi", pghi_in)):
                        t = pidxp.tile([128, PH], FP32, tag=nm)
                        nc.sync.dma_start(t[:], apin[:])
                        pg_t[nm] = t

                for hh in range(2):
                    wh = W_LO if hh == 0 else W_HI
                    tabt = tabp.tile([128, wh + 1], FP32, tag="dtab")
                    nc.vector.memset(tabt[:, wh:wh + 1], 0.0)
                    base = hh * HALF1
                    for g in range(8):
                        col = 0
                        while col < wh:
                            node = base + col
                            shard = node // NS
                            off = node % NS
                            ln = min(NS - off, wh - col)
                            nc.sync.dma_start(
                                tabt[16 * g:16 * g + 16, col:col + ln],
                                cc_out[16 * shard:16 * shard + 16, off:off + ln])
                            col += ln
                    dnm = "pdlo" if hh == 0 else "pdhi"
                    snm = "pslo" if hh == 0 else "pshi"
                    for q in range(NQ_CALLS):
                        qs = q * (PQ // 16)
                        sd = goutp.tile([128, PQ], FP32, tag="sd")
                        nc.gpsimd.ap_gather(sd[:], tabt[:],
                                            pidx_t[dnm][:, qs:qs + PQ // 16],
                                            128, wh + 1, 1, PQ)
                        ss = goutp.tile([128, PQ], FP32, tag="ss")
                        nc.gpsimd.ap_gather(ss[:], srctab[:],
                                            pidx_t[snm][:, qs:qs + PQ // 16],
                                            128, NS + 1, 1, PQ)
                        if not gv_ones:
                            gnm = "pglo" if hh == 0 else "pghi"
                            nc.vector.tensor_tensor(
                                sd[:], sd[:], pg_t[gnm][:, q * PQ:(q + 1) * PQ],
                                mybir.AluOpType.mult)
                        prod = ewp.tile([128, PQ], FP32, tag="prod")
                        nc.vector.tensor_tensor(prod[:], sd[:], ss[:],
                                                mybir.AluOpType.mult)
                        tmp1 = ewp.tile([128, 1], FP32, tag="tmp1")
                        nc.vector.tensor_reduce(tmp1[:], prod[:],
                                                mybir.AxisListType.X,
                                                mybir.AluOpType.add)
                        nc.vector.tensor_tensor(edge_acc[:], edge_acc[:],
                                                tmp1[:], mybir.AluOpType.add)
                        tmp2 = ewp.tile([128, 1], FP32, tag="tmp2")
                        nc.vector.tensor_reduce(tmp2[:], sd[:],
                                                mybir.AxisListType.X,
                                                mybir.AluOpType.add)
                        nc.vector.tensor_tensor(nl_acc[:], nl_acc[:],
                                                tmp2[:], mybir.AluOpType.add)

                # ---------- Phase F: partials -> AllReduce -> scalar ----------
                with tc.tile_pool(name="psF", bufs=1, space="PSUM") as psFp:
                    pe1 = psFp.tile([1, 1], FP32, tag="pe1")
                    nc.tensor.matmul(pe1[:], edge_acc[:], ones_sb[:],
                                     start=True, stop=True)
                    pn = psFp.tile([1, K], FP32, tag="pn")
                    nc.tensor.matmul(pn[:], nl_acc[:], grp_sb[:],
                                     start=True, stop=True)
                    pc = psFp.tile([1, K], FP32, tag="pc")
                    nc.tensor.matmul(pc[:], ones_sb[:], cs_acc[:],
                                     start=True, stop=True)
                    vec = ewp.tile([1, 33], FP32, tag="vec")
                    nc.scalar.activation(vec[:, 0:1], pe1[:],
                                         mybir.ActivationFunctionType.Copy)
                    nc.scalar.activation(vec[:, 1:17], pn[:],
                                         mybir.ActivationFunctionType.Copy)
                    nc.scalar.activation(vec[:, 17:33], pc[:],
                                         mybir.ActivationFunctionType.Copy)
                    nc.sync.dma_start(cc2_in[:], vec[:])
                    tc.strict_bb_all_engine_barrier()
                    nc.gpsimd.collective_compute(
                        "AllReduce", mybir.AluOpType.add,
                        ins=[cc2_in[:]], outs=[cc2_out[:]], replica_groups=rg)
                    tc.strict_bb_all_engine_barrier()

                    fin = ewp.tile([1, 33], FP32, tag="fin")
                    nc.sync.dma_start(fin[:], cc2_out[:])
                    t16 = ewp.tile([1, K], FP32, tag="t16")
                    nc.vector.tensor_tensor(t16[:], fin[:, 1:17], fin[:, 1:17],
                                            mybir.AluOpType.mult)
                    r1 = ewp.tile([1, 1], FP32, tag="r1")
                    nc.vector.tensor_reduce(r1[:], t16[:],
                                            mybir.AxisListType.X,
                                            mybir.AluOpType.add)
                    t16b = ewp.tile([1, K], FP32, tag="t16b")
                    nc.vector.tensor_tensor(t16b[:], fin[:, 17:33],
                                            fin[:, 17:33],
                                            mybir.AluOpType.mult)
                    r2 = ewp.tile([1, 1], FP32, tag="r2")
                    nc.vector.tensor_reduce(r2[:], t16b[:],
                                            mybir.AxisListType.X,
                                            mybir.AluOpType.add)
                    ncs = ewp.tile([1, 1], FP32, tag="ncs")
                    nc.scalar.activation(ncs[:], r2[:],
                                         mybir.ActivationFunctionType.Sqrt)
                    aa = ewp.tile([1, 1], FP32, tag="aa")
                    nc.vector.tensor_scalar(aa[:], fin[:, 0:1],
                                            -1.0 / (2.0 * E), None,
                                            mybir.AluOpType.mult)
                    bb2 = ewp.tile([1, 1], FP32, tag="bb2")
                    nc.vector.tensor_scalar(bb2[:], r1[:],
                                            1.0 / (4.0 * E * E), None,
                                            mybir.AluOpType.mult)
                    sp = ewp.tile([1, 1], FP32, tag="sp")
                    nc.vector.tensor_tensor(sp[:], aa[:], bb2[:],
                                            mybir.AluOpType.add)
                    cl = ewp.tile([1, 1], FP32, tag="cl")
                    nc.vector.tensor_scalar(cl[:], ncs[:],
                                            float(np.sqrt(K)) / N, 1.0,
                                            mybir.AluOpType.mult,
                                            mybir.AluOpType.subtract)
                    yv = ewp.tile([1, 1], FP32, tag="yv")
                    nc.vector.tensor_tensor(yv[:], sp[:], cl[:],
                                            mybir.AluOpType.add)
                    nc.sync.dma_start(y_out[:], yv[:])

    nc.compile()
    return nc


def kernel(src, dst, graph_vals, gnorm_vals, features, W, b, Wt, bt):
    global _last_results
    in_maps, CH, PH, gv_ones = _prep_inputs(
        src, dst, graph_vals, gnorm_vals, features, W, b, Wt, bt)
    nc = _build_module(CH, PH, gv_ones)
    res = run_bass_kernel_spmd(nc, in_maps, core_ids=list(range(NCORES)))
    _last_results = res
    out = res.results[0]["y"]
    return np.asarray(out, dtype=np.float32).reshape(())
